# revision 7
# baseline (speedup 1.0000x reference)
"""Branching-Kriging pairwise kernel matrix on 8 Trainium2 NeuronCores.

Math: for rows i of W1 and j of W2,
    K(i,j) = exp(share_k + branch_k + nested_k)
Every term is a sum over products of a function of i and a function of j
(the categorical branch/level structure is one-hot encodable), so
    log K = F1 @ F2.T
with F1 [4096, 79] and F2 [2048, 79] feature matrices (padded to 128).
The device kernel is a K=128 fp32 matmul + ACT exp + 32 MiB output
write, sharded along n1 (rows of W1) across the 8 cores.
"""

import numpy as np

import concourse.bacc as bacc
import concourse.bass as bass
import concourse.mybir as mybir
import concourse.tile as tile
from concourse._compat import get_trn_type
from concourse.bass_utils import run_bass_kernel_spmd

N_CORES = 8
N1, N2 = 4096, 2048
ROWS = N1 // N_CORES          # 512 output rows per core
D = 128                       # feature (contraction) dim, padded from 79
S, B = 8, 3                   # spatial / branching factor counts
NEST = [3, 3, 3]              # nested factors per branching factor

FP32 = mybir.dt.float32
FP32R = mybir.dt.float32r


def _act(x):
    return np.minimum(np.where(x >= 0.0, x + 1.0, np.exp(x)), 30.0).astype(np.float32)


def _build_features(W1, W2, alpha, theta, gamma0, gamma1, gamma2):
    """log K = F1 @ F2.T, exactly (up to fp32 rounding)."""
    W1 = np.asarray(W1, np.float32)
    W2 = np.asarray(W2, np.float32)
    n1, n2 = W1.shape[0], W2.shape[0]
    X1, Z1, V1 = W1[:, :S], W1[:, S:S + B], W1[:, S + B:]
    X2, Z2, V2 = W2[:, :S], W2[:, S:S + B], W2[:, S + B:]
    a = _act(np.asarray(alpha))[0]            # [S]
    t = _act(np.asarray(theta))[0]            # [B]
    G = [_act(np.asarray(g)) - 1.0 for g in (gamma0, gamma1, gamma2)]  # [nb, 4]

    F1 = np.zeros((n1, D), np.float32)
    F2 = np.zeros((n2, D), np.float32)

    # row terms + constant
    F1[:, 0] = 1.0
    F2[:, 0] = -(X2**2 @ a) - (V2**2).sum(1) - t.sum()
    F1[:, 1] = -(X1**2 @ a) - (V1**2).sum(1)
    F2[:, 1] = 1.0
    # share cross: 2 a_s x1 x2
    F1[:, 2:10] = 2.0 * a[None, :] * X1
    F2[:, 2:10] = X2
    # nested v cross (level-independent part): 2 v1 v2
    F1[:, 10:19] = 2.0 * V1
    F2[:, 10:19] = V2

    d = 19
    Z1i = Z1.astype(np.int32)
    Z2i = Z2.astype(np.int32)
    off = 0
    for b in range(B):
        nb = NEST[b]
        v1b = V1[:, off:off + nb]
        v2b = V2[:, off:off + nb]
        for lev in range(1, 5):
            e1 = (Z1i[:, b] == lev).astype(np.float32)
            e2 = (Z2i[:, b] == lev).astype(np.float32)
            g = G[b][:, lev - 1]
            # branch match reward t_b, minus gamma-weighted v2^2
            F1[:, d] = e1
            F2[:, d] = e2 * (t[b] - (v2b**2) @ g)
            d += 1
            # gamma-weighted v1^2
            F1[:, d] = -e1 * ((v1b**2) @ g)
            F2[:, d] = e2
            d += 1
            # gamma-weighted cross terms
            F1[:, d:d + nb] = 2.0 * e1[:, None] * v1b * g[None, :]
            F2[:, d:d + nb] = e2[:, None] * v2b
            d += nb
        off += nb
    assert d == 79

    # The PE's fp32r matmul rounds operands to ~12-bit mantissa. Pre-round
    # both feature matrices so the hardware rounding is a no-op, then spend
    # the spare contraction dims (79..118) on residual-correction columns
    # for the worst error contributors: F*G = r(F)r(G) + L_F r(G) + r(F) L_G
    # up to a negligible L_F*L_G term.
    def _r12(x):
        m, e = np.frexp(x)
        return (np.round(m * 4096.0) / 4096.0 * 2.0**e).astype(np.float32)

    nd = d
    L1 = F1[:, :nd] - _r12(F1[:, :nd])
    L2 = F2[:, :nd] - _r12(F2[:, :nd])
    c1 = np.abs(L1).max(0) * np.abs(F2[:, :nd]).max(0)
    c2 = np.abs(F1[:, :nd]).max(0) * np.abs(L2).max(0)
    cand = [(c1[i], i, 1) for i in range(nd)] + [(c2[i], i, 2) for i in range(nd)]
    cand.sort(key=lambda t: -t[0])
    F1[:, :nd] = _r12(F1[:, :nd])
    F2[:, :nd] = _r12(F2[:, :nd])
    for c, i, side in cand[:min(D - nd, 40)]:
        if c <= 0.0:
            break
        if side == 1:
            F1[:, d] = _r12(L1[:, i])
            F2[:, d] = F2[:, i]
        else:
            F1[:, d] = F1[:, i]
            F2[:, d] = _r12(L2[:, i])
        d += 1
    return F1, F2


_COMPILED = None


def _get_nc():
    global _COMPILED
    if _COMPILED is not None:
        return _COMPILED

    nc = bacc.Bacc(get_trn_type(), target_bir_lowering=False, debug=False)
    f1t = nc.dram_tensor("f1t", [D, ROWS], FP32R, kind="ExternalInput")
    f2t = nc.dram_tensor("f2t", [D, N2], FP32R, kind="ExternalInput")
    out = nc.dram_tensor("out", [ROWS, N2], FP32, kind="ExternalOutput")

    MT = ROWS // 128          # 4 output row tiles per core
    NT = N2 // 512            # 4 output col tiles

    H = N2 // 2               # 1024: half-width for exp/store granularity

    with tile.TileContext(nc) as tc:
        with (
            tc.tile_pool(name="inp", bufs=1) as inp,
            tc.tile_pool(name="ps", bufs=2, space=bass.MemorySpace.PSUM) as psp,
            tc.tile_pool(name="ob", bufs=4) as obp,
        ):
            f1s = inp.tile([D, ROWS], FP32R, tag="f1")
            f2sa = inp.tile([D, H], FP32R, tag="f2a")
            f2sb = inp.tile([D, H], FP32R, tag="f2b")
            # each dma_start costs ~0.6us of serialized sequencer issue time;
            # split f2 once so the first matmuls only wait on f1 + half of f2
            nc.sync.dma_start(f1s[:], f1t[:])
            nc.sync.dma_start(f2sa[:], f2t[:, :H])
            nc.sync.dma_start(f2sb[:], f2t[:, H:])
            for mt in range(MT):
                ps = psp.tile([128, N2], FP32, tag="ps")  # 4 PSUM banks
                for nt in range(NT):
                    f2h = f2sa if nt < 2 else f2sb
                    nc.tensor.matmul(
                        ps[:, nt * 512:(nt + 1) * 512],
                        f1s[:, mt * 128:(mt + 1) * 128],
                        f2h[:, (nt % 2) * 512:(nt % 2 + 1) * 512],
                        start=True,
                        stop=True,
                    )
                for h in range(2):
                    ot = obp.tile([128, H], FP32, tag=f"ot{h}")
                    nc.scalar.activation(
                        ot[:], ps[:, h * H:(h + 1) * H],
                        mybir.ActivationFunctionType.Exp,
                    )
                    nc.sync.dma_start(
                        out[mt * 128:(mt + 1) * 128, h * H:(h + 1) * H], ot[:]
                    )

    nc.compile()
    _COMPILED = nc
    return _COMPILED


LAST_RESULTS = None


def _ensure_ntff_hook():
    """The agent image's `antenv` lacks `axon_hooks`; register the
    boot-shipped ctypes NTFF hook under that name so trace=True works."""
    import sys
    import types

    try:
        import antenv.axon_hooks  # noqa: F401
        return
    except ImportError:
        pass
    mod = types.ModuleType("antenv.axon_hooks")
    mod._hook = None

    def set_axon_ntff_profile_hook(hook):
        mod._hook = hook

    def get_axon_ntff_profile_hook():
        return mod._hook

    mod.set_axon_ntff_profile_hook = set_axon_ntff_profile_hook
    mod.get_axon_ntff_profile_hook = get_axon_ntff_profile_hook
    sys.modules["antenv.axon_hooks"] = mod
    import antenv

    antenv.axon_hooks = mod
    try:
        from trn_agent_boot.trn_boot import _ntff_profile_via_ctypes

        mod._hook = _ntff_profile_via_ctypes("/opt/axon/libaxon_pjrt.so")
    except Exception:
        pass
    # artifact upload needs bucket creds this container may not have;
    # the local NTFF -> perfetto pipeline doesn't depend on it
    import concourse.bass_utils as _bu

    _orig_upload = _bu.upload_artifacts

    def _safe_upload(tmpdir):
        try:
            return _orig_upload(tmpdir)
        except Exception:
            return tmpdir

    _bu.upload_artifacts = _safe_upload


def kernel(W1, W2, alpha, theta, gamma0, gamma1, gamma2, _profile=False):
    global LAST_RESULTS
    if _profile:
        _ensure_ntff_hook()
    F1, F2 = _build_features(W1, W2, alpha, theta, gamma0, gamma1, gamma2)
    f1t = np.ascontiguousarray(F1.T)      # [D, N1]
    f2t = np.ascontiguousarray(F2.T)      # [D, N2]
    in_maps = [
        {
            "f1t": np.ascontiguousarray(f1t[:, c * ROWS:(c + 1) * ROWS]),
            "f2t": f2t,
        }
        for c in range(N_CORES)
    ]
    nc = _get_nc()
    res = run_bass_kernel_spmd(nc, in_maps, list(range(N_CORES)), trace=_profile)
    LAST_RESULTS = res
    return np.concatenate(
        [res.results[c]["out"] for c in range(N_CORES)], axis=0
    )
